# revision 1
# baseline (speedup 1.0000x reference)
"""Fixed-point attention (nn_Attention_17918603559191) on 8 TRN2 NeuronCores.

Sharding (no collectives): core c -> batch b = c//2, token-half t = c%2.
Each core computes K,V for all 2048 tokens of its batch (25% redundant
FLOPs on the qkv stage) and Q/attention/proj for its 1024 tokens. The
host rotates the token axis of x per core so that the core's q-tokens
are always columns 0..1023 (keeps the SPMD program identical across
cores); attention is invariant to permuting the key/value axis.

Numerics (validated vs reference in fp64/numpy, rel ~8e-3):
 - all matmuls fp16 (1 cyc/row on PE), fp32 PSUM accumulation
 - weights pre-scaled by 4096 on host so PSUM results are in "grid units"
 - make_fxp = clip to +-32767, signed-trunc via ACT Sign + DVE
   round-to-nearest int16 cast with a 1e-3 guard
 - dots-fxp step skipped (numerically negligible, validated)
 - attn = floor(exp * 4096/S): exp on ACT (fp16), S via PE ones-matmul
   over the partition axis, 1/S broadcast via stride-0 DMA,
   floor via (y*4096/S) - 0.499 -> int16 -> fp16
"""

import sys

sys.path.insert(0, "/opt/trn_rl_repo")

import numpy as np
import concourse.bass as bass
import concourse.tile as tile
from concourse import mybir, bacc
from concourse.bass_utils import run_bass_kernel_spmd

F32 = mybir.dt.float32
F16 = mybir.dt.float16
I16 = mybir.dt.int16
ALU = mybir.AluOpType
AF = mybir.ActivationFunctionType

D = 1024      # model dim
M = 2048      # key/value tokens per core (full batch)
NQ = 1024     # query tokens per core
H = 16
DH = 64
HP = H // 2   # head pairs
GRID = 4096.0

_CACHED_NC = None


def _rep_free(ap, g):
    """Repeat a [P, W] AP g times along a new middle free dim (stride 0)."""
    return bass.AP(tensor=ap.tensor, offset=ap.offset,
                   ap=[ap.ap[0], [0, g], ap.ap[1]])


def _bcast_part(ap, p=128):
    """Broadcast a [1, W] AP across p partitions (stride 0 partition dim)."""
    return bass.AP(tensor=ap.tensor, offset=ap.offset,
                   ap=[[0, p], ap.ap[-1]])


def _fxp_chain(nc, pools, src_psum, dst_f16_ap, bias_ap=None, out_f32_ap=None):
    """dst = signed_trunc(clip(src [+bias], +-32767)) / 4096 as fp16 (or f32).

    src_psum: [128, W] fp32 PSUM AP (grid units). One of dst_f16_ap /
    out_f32_ap receives the result in natural units.
    """
    p, w = src_psum.shape[0], src_psum.free_size()
    t = pools["fx_t"].tile([p, w], F32, tag="fx_t")
    if bias_ap is not None:
        nc.vector.tensor_scalar(t, src_psum, bias_ap, 32767.0,
                                op0=ALU.add, op1=ALU.min)
        nc.vector.tensor_scalar(t, t, -32767.0, None, op0=ALU.max)
    else:
        nc.vector.tensor_scalar(t, src_psum, 32767.0, -32767.0,
                                op0=ALU.min, op1=ALU.max)
    s = pools["fx_s"].tile([p, w], F16, tag="fx_s")
    nc.scalar.activation(s, t, AF.Sign)  # {-1, 0, +1}
    gi = pools["fx_i"].tile([p, w], I16, tag="fx_i")
    nc.vector.scalar_tensor_tensor(gi, in0=s, scalar=-0.499, in1=t,
                                   op0=ALU.mult, op1=ALU.add)
    if dst_f16_ap is not None:
        nc.vector.tensor_scalar(dst_f16_ap, gi, 1.0 / GRID, None, op0=ALU.mult)
    if out_f32_ap is not None:
        nc.vector.tensor_scalar(out_f32_ap, gi, 1.0 / GRID, None, op0=ALU.mult)


def build_kernel():
    nc = bacc.Bacc(name="fxp_attn")
    xT_e = nc.declare_dram_parameter("xT", [D, M], F32, isOutput=False)
    wqkvT_e = nc.declare_dram_parameter("wqkvT", [D, 3 * D], F32, isOutput=False)
    wprojT_e = nc.declare_dram_parameter("wprojT", [D, D], F32, isOutput=False)
    bias_e = nc.declare_dram_parameter("bias", [1, D], F32, isOutput=False)
    out_e = nc.declare_dram_parameter("out", [D, NQ], F32, isOutput=True)

    with tile.TileContext(nc) as tc:
        from contextlib import ExitStack
        with ExitStack() as ctx:
            persist = ctx.enter_context(tc.tile_pool(name="persist", bufs=1))
            fxp = {
                "fx_t": ctx.enter_context(tc.tile_pool(name="fx_t", bufs=2)),
                "fx_s": ctx.enter_context(tc.tile_pool(name="fx_s", bufs=2)),
                "fx_i": ctx.enter_context(tc.tile_pool(name="fx_i", bufs=2)),
            }

            ones = persist.tile([128, 1], F16, tag="ones")
            nc.vector.memset(ones, 1.0)
            ones_row = persist.tile([1, 128], F16, tag="ones_row")
            nc.vector.memset(ones_row, 1.0)
            bias_sb = persist.tile([128, 8], F32, tag="bias")
            nc.sync.dma_start(out=bias_sb, in_=bass.AP(
                tensor=bias_e.ap().tensor, offset=0, ap=[[1, 128], [128, 8]]))

            # persistent fp16 activations
            k_s = [persist.tile([128, M], F16, tag=f"k{s}", name=f"k{s}") for s in range(8)]
            q_s = [persist.tile([128, NQ], F16, tag=f"q{s}", name=f"q{s}") for s in range(8)]
            v_t = [persist.tile([128, D], F16, tag=f"v{t}", name=f"v{t}") for t in range(16)]
            wp_s = [persist.tile([128, D], F16, tag=f"wp{s}", name=f"wp{s}") for s in range(8)]

            # ---------------- Phase 0/1: load inputs, qkv matmuls ----------
            with tc.tile_pool(name="ph1", bufs=1) as ph1, \
                 tc.tile_pool(name="stg", bufs=2) as stg, \
                 tc.tile_pool(name="wstg", bufs=2) as wstg, \
                 tc.tile_pool(name="ps1", bufs=3, space="PSUM") as ps1:

                # xT fp16, resident: [128, 8, 2048]
                xT = ph1.tile([128, 8, M], F16, tag="xT")
                for s in range(8):
                    st = stg.tile([128, M], F32, tag="stg")
                    nc.sync.dma_start(out=st, in_=xT_e.ap()[s * 128:(s + 1) * 128, :])
                    nc.scalar.copy(xT[:, s, :], st)

                # w_v fp16 resident: wqkvT cols [2048:3072] -> [128, 8, 1024]
                wv = ph1.tile([128, 8, D], F16, tag="wv")
                for quart in range(4):
                    st = stg.tile([128, M], F32, tag="stg")
                    st3 = bass.AP(tensor=st.tensor, offset=st.offset,
                                  ap=[st.ap[0], [256, 8], [1, 256]])
                    nc.sync.dma_start(out=st3, in_=bass.AP(
                        tensor=wqkvT_e.ap().tensor,
                        offset=2 * D + quart * 256,
                        ap=[[3 * D, 128], [3 * D * 128, 8], [1, 256]]))
                    nc.scalar.copy(wv[:, :, quart * 256:(quart + 1) * 256], st3)

                # w_proj fp16 resident (8 e-strips x [128, 1024])
                for s in range(8):
                    st = stg.tile([128, D], F32, tag="stg")
                    nc.sync.dma_start(out=st, in_=wprojT_e.ap()[s * 128:(s + 1) * 128, :])
                    nc.scalar.copy(wp_s[s], st)

                def stream_w(col0, ncols):
                    """DMA wqkvT[:, col0:col0+ncols] -> f16 [128, 8, ncols]."""
                    st = wstg.tile([128, 8, 128], F32, tag="wstg")
                    wf = wstg.tile([128, 8, 128], F16, tag="wstg16")
                    nc.sync.dma_start(out=st[:, :, :ncols], in_=bass.AP(
                        tensor=wqkvT_e.ap().tensor, offset=col0,
                        ap=[[3 * D, 128], [3 * D * 128, 8], [1, ncols]]))
                    nc.scalar.copy(wf[:, :, :ncols], st[:, :, :ncols])
                    return wf

                # K: feature-major [kv-dim strip s][128, 2048]
                for s in range(8):
                    wf = stream_w(D + s * 128, 128)
                    for mc in range(4):
                        pt = ps1.tile([128, 512], F32, tag="ps1")
                        for dt in range(8):
                            nc.tensor.matmul(
                                pt, lhsT=wf[:, dt, :],
                                rhs=xT[:, dt, mc * 512:(mc + 1) * 512],
                                start=(dt == 0), stop=(dt == 7))
                        _fxp_chain(nc, fxp, pt[:, :],
                                   k_s[s][:, mc * 512:(mc + 1) * 512])

                # Q: feature-major, tokens 0..1023 of rotated xT
                for s in range(8):
                    wf = stream_w(s * 128, 128)
                    for mc in range(2):
                        pt = ps1.tile([128, 512], F32, tag="ps1")
                        for dt in range(8):
                            nc.tensor.matmul(
                                pt, lhsT=wf[:, dt, :],
                                rhs=xT[:, dt, mc * 512:(mc + 1) * 512],
                                start=(dt == 0), stop=(dt == 7))
                        _fxp_chain(nc, fxp, pt[:, :],
                                   q_s[s][:, mc * 512:(mc + 1) * 512])

                # V: token-major [tok strip ts][128, 1024]
                for ts in range(16):
                    for cc in range(2):
                        pt = ps1.tile([128, 512], F32, tag="ps1")
                        for dt in range(8):
                            nc.tensor.matmul(
                                pt, lhsT=xT[:, dt, ts * 128:(ts + 1) * 128],
                                rhs=wv[:, dt, cc * 512:(cc + 1) * 512],
                                start=(dt == 0), stop=(dt == 7))
                        _fxp_chain(nc, fxp, pt[:, :],
                                   v_t[ts][:, cc * 512:(cc + 1) * 512])

            # ---------------- Phase 2: attention ---------------------------
            NCH = 2   # n-chunks of 512
            aop = ctx.enter_context(tc.tile_pool(name="aop", bufs=1))
            ao_s = [aop.tile([128, NQ], F16, tag=f"ao{s}", name=f"ao{s}")
                    for s in range(8)]
            with tc.tile_pool(name="expp", bufs=1) as expp, \
                 tc.tile_pool(name="attn", bufs=2) as attnp, \
                 tc.tile_pool(name="rbp", bufs=1) as rbp, \
                 tc.tile_pool(name="dots", bufs=1, space="PSUM") as dotsp, \
                 tc.tile_pool(name="spsum", bufs=1, space="PSUM") as spsum, \
                 tc.tile_pool(name="avps", bufs=1, space="PSUM") as avps:

                for hp in range(HP):
                    for ch in range(NCH):
                        n0 = ch * 512
                        # --- dots + exp, 2 m-tiles per PSUM supertile ---
                        exp_b = [expp.tile([128, 16, 512], F16, tag=f"exp{h}", name=f"exp{h}")
                                 for h in range(2)]
                        for g in range(8):
                            dt_ps = [dotsp.tile([128, 2, 512], F32, tag=f"dots{h}", name=f"dots{h}")
                                     for h in range(2)]
                            for sub in range(2):
                                mt = 2 * g + sub
                                for h in range(2):
                                    p0 = h * 64
                                    nc.tensor.matmul(
                                        dt_ps[h][:, sub, :],
                                        lhsT=k_s[hp][p0:p0 + 64,
                                                     mt * 128:(mt + 1) * 128],
                                        rhs=q_s[hp][p0:p0 + 64, n0:n0 + 512],
                                        start=True, stop=True,
                                        tile_position=(p0, 0))
                            for h in range(2):
                                nc.scalar.activation(
                                    exp_b[h][:, 2 * g:2 * g + 2, :],
                                    dt_ps[h][:, :, :], AF.Exp, scale=0.125)

                        # --- S = sum_m exp via ones-matmul, then 1/S ---
                        rb = []
                        for h in range(2):
                            S = spsum.tile([1, 512], F32, tag=f"S{h}")
                            for mt in range(16):
                                nc.tensor.matmul(
                                    S, lhsT=ones, rhs=exp_b[h][:, mt, :],
                                    start=(mt == 0), stop=(mt == 15))
                            rS = rbp.tile([1, 512], F16, tag=f"rS{h}")
                            with nc.allow_low_precision(reason="1/S in fp16 validated numerically"):
                                nc.vector.reciprocal(rS, S)
                            rb_ps = dotsp.tile([128, 2, 512], F32,
                                               tag=f"dots{h}", name=f"rbps{h}")
                            nc.tensor.matmul(rb_ps[:, 0, :], lhsT=ones_row,
                                             rhs=rS, start=True, stop=True)
                            rbt = rbp.tile([128, 512], F16, tag=f"rb{h}")
                            nc.vector.tensor_copy(rbt, rb_ps[:, 0, :])
                            rb.append(rbt)

                        # --- attn = floor(exp*4096/S) -> fp16; attn @ v ---
                        av0 = avps.tile([128, 512], F32, tag="av0", bufs=1)
                        av1 = avps.tile([128, 512], F32, tag="av1", bufs=1)
                        avt = [av0, av1]
                        for g in range(4):
                            for h in range(2):
                                y = attnp.tile([128, 4, 512], F16, tag="y", bufs=1)
                                nc.vector.scalar_tensor_tensor(
                                    y, in0=exp_b[h][:, 4 * g:4 * g + 4, :],
                                    scalar=GRID, in1=_rep_free(rb[h][:, :], 4),
                                    op0=ALU.mult, op1=ALU.mult)
                                ai = attnp.tile([128, 4, 512], I16, tag="ai")
                                nc.vector.tensor_scalar(
                                    ai, y, -0.499, None, op0=ALU.add)
                                af = attnp.tile([128, 4, 512], F16, tag="af")
                                nc.gpsimd.tensor_copy(af, ai)
                                for sub in range(4):
                                    mt = 4 * g + sub
                                    p0 = h * 64
                                    nc.tensor.matmul(
                                        avt[h][p0:p0 + 64, :],
                                        lhsT=v_t[mt][:, (2 * hp + h) * 64:
                                                     (2 * hp + h + 1) * 64],
                                        rhs=af[:, sub, :],
                                        start=(mt == 0), stop=(mt == 15),
                                        tile_position=(0, p0))
                        _fxp_chain(nc, fxp, av0[0:64, :],
                                   ao_s[hp][0:64, n0:n0 + 512])
                        _fxp_chain(nc, fxp, av1[64:128, :],
                                   ao_s[hp][64:128, n0:n0 + 512])

            # ---------------- Phase 3: projection --------------------------
            with tc.tile_pool(name="ps3", bufs=2, space="PSUM") as ps3, \
                 tc.tile_pool(name="outp", bufs=2) as outp:
                for ds in range(8):
                    for ch in range(2):
                        pt = ps3.tile([128, 512], F32, tag="ps3")
                        for es in range(8):
                            nc.tensor.matmul(
                                pt, lhsT=wp_s[es][:, ds * 128:(ds + 1) * 128],
                                rhs=ao_s[es][:, ch * 512:(ch + 1) * 512],
                                start=(es == 0), stop=(es == 7))
                        ot = outp.tile([128, 512], F32, tag="ot")
                        _fxp_chain(nc, fxp, pt[:, :], None,
                                   bias_ap=bias_sb[:, ds:ds + 1],
                                   out_f32_ap=ot[:, :])
                        nc.sync.dma_start(
                            out=out_e.ap()[ds * 128:(ds + 1) * 128,
                                           ch * 512:(ch + 1) * 512],
                            in_=ot)

    nc.compile()
    return nc


def _get_nc():
    global _CACHED_NC
    if _CACHED_NC is None:
        _CACHED_NC = build_kernel()
    return _CACHED_NC


def kernel(x, w_qkv, w_proj, b_proj, **_):
    B, N, Dm = x.shape
    assert (B, N, Dm) == (4, 2048, 1024)
    nc = _get_nc()

    wqkvT = np.ascontiguousarray((w_qkv.astype(np.float32) * GRID).T)
    wprojT = np.ascontiguousarray((w_proj.astype(np.float32) * GRID).T)
    bias = (b_proj.astype(np.float32) * GRID).reshape(1, D)

    in_maps = []
    for c in range(8):
        b, t = c // 2, c % 2
        xb = x[b].astype(np.float32)
        xrot = np.concatenate([xb[t * NQ:], xb[:t * NQ]], axis=0)
        in_maps.append({
            "xT": np.ascontiguousarray(xrot.T),
            "wqkvT": wqkvT,
            "wprojT": wprojT,
            "bias": bias,
        })

    res = run_bass_kernel_spmd(nc, in_maps, list(range(8)))
    global LAST_RESULT
    LAST_RESULT = res
    out = np.empty((B, N, Dm), dtype=np.float32)
    for c in range(8):
        b, t = c // 2, c % 2
        out[b, t * NQ:(t + 1) * NQ, :] = res.results[c]["out"].T
    return out



# revision 14
# speedup vs baseline: 1.9656x; 1.9656x over previous
"""Fixed-point attention (nn_Attention_17918603559191) on 8 TRN2 NeuronCores.

Sharding (no collectives): core c -> batch b = c//2, token-half t = c%2.
Each core computes K,V for all 2048 tokens of its batch (25% redundant
FLOPs on the qkv stage) and Q/attention/proj for its 1024 tokens. The
host rotates the token axis of x per core so that the core's q-tokens
are always columns 0..1023 (keeps the SPMD program identical across
cores); attention is invariant to permuting the key/value axis.

v2 numerics/pipeline (validated vs reference, rel ~9e-3):
 - host pre-converts all inputs to fp16 (weights pre-scaled by 4096 so
   PSUM results are in grid units); single large DMA per tensor
 - q/k/v: PSUM -> fp16 via one ACT Copy(scale=1/4096); the exact
   4096-grid truncation is skipped (validated: |q|max < 6 so no clip,
   trunc-vs-round differences are ~2e-4 relative and wash out)
 - S = sum_m exp: mt-slabs 0..NS_PE-1 accumulate via ones[128,128]
   matmul into PSUM; remaining slabs are mt-folded on DVE
   (tensor_reduce over a strided view) and folded into the same PSUM
   with one more accumulating ones-matmul. 4096/S via
   reciprocal_approx_fast (fp32) + ACT scale-copy.
 - attn = floor(exp * 4096/S): DVE tensor_tensor (y, f16), DVE
   tensor_scalar -0.499 -> int16 (round-to-nearest == floor), then the
   int16 -> fp16 cast split between DVE (h=0) and ACT (h=1) to balance
   engine load. No GPSIMD.
 - output returned as int16 grid units; host divides by 4096
"""

import sys

sys.path.insert(0, "/opt/trn_rl_repo")

import numpy as np
import concourse.bass as bass
import concourse.tile as tile
from concourse import mybir, bacc
from concourse.bass_utils import run_bass_kernel_spmd

F32 = mybir.dt.float32
F16 = mybir.dt.float16
I16 = mybir.dt.int16
ALU = mybir.AluOpType
AF = mybir.ActivationFunctionType

D = 1024      # model dim
M = 2048      # key/value tokens per core (full batch)
NQ = 1024     # query tokens per core
H = 16
DH = 64
HP = H // 2   # head pairs
GRID = 4096.0
NS_PE = 6     # mt-slabs of the softmax denominator summed on PE (rest DVE)

_CACHED_NC = None


def _rep_free(ap, g):
    """Repeat a [P, W] AP g times along a new middle free dim (stride 0)."""
    return bass.AP(tensor=ap.tensor, offset=ap.offset,
                   ap=[ap.ap[0], [0, g], ap.ap[1]])


def build_kernel():
    nc = bacc.Bacc(name="fxp_attn")
    # all activations/weights fp16, host-prepared layouts:
    #   xT   [128, 8, 2048]  xT[p, dt, m] = x_rot[m, dt*128+p] (grid-neutral)
    #   wq   [128, 8, 1024]  wq[p, dt, j] = 4096*w_qkv[j, dt*128+p]
    #   wk   [128, 8, 1024]  cols 1024:2048 of qkv
    #   wv   [128, 8, 1024]  cols 2048:3072
    #   wp   [128, 8, 1024]  wp[p, es, j] = 4096*w_proj[j, es*128+p]
    #   bias [128, 8] f32    bias[p, ds] = 4096*b_proj[ds*128+p]
    xT_e = nc.declare_dram_parameter("xT", [128, 8 * M], F16, isOutput=False)
    wq_e = nc.declare_dram_parameter("wq", [128, 8 * D], F16, isOutput=False)
    wk_e = nc.declare_dram_parameter("wk", [128, 8 * D], F16, isOutput=False)
    wv_e = nc.declare_dram_parameter("wv", [128, 8 * D], F16, isOutput=False)
    wp_e = nc.declare_dram_parameter("wp", [128, 8 * D], F16, isOutput=False)
    bias_e = nc.declare_dram_parameter("bias", [128, 8], F32, isOutput=False)
    # out[p, ds*1024 + n] = int16 grid value of out[ds*128+p, n]
    out_e = nc.declare_dram_parameter("out", [128, 8 * NQ], I16, isOutput=True)

    def r3(ap, a, b):
        """View a [128, a*b] DRAM AP as [128, a, b]."""
        return bass.AP(tensor=ap.tensor, offset=0,
                       ap=[ap.ap[0], [b, a], [1, b]])

    with tile.TileContext(nc) as tc:
        from contextlib import ExitStack
        with ExitStack() as ctx:
            persist = ctx.enter_context(tc.tile_pool(name="persist", bufs=1))

            ones2d = persist.tile([128, 128], F16, tag="ones2d")
            nc.vector.memset(ones2d, 1.0)
            bias_sb = persist.tile([128, 8], F32, tag="bias")
            nc.sync.dma_start(out=bias_sb, in_=bias_e.ap())

            # persistent fp16 activations
            k_s = [persist.tile([128, M], F16, tag=f"k{s}", name=f"k{s}") for s in range(8)]
            q_s = [persist.tile([128, NQ], F16, tag=f"q{s}", name=f"q{s}") for s in range(8)]
            v_t = [persist.tile([128, D], F16, tag=f"v{t}", name=f"v{t}") for t in range(16)]
            wp_sb = persist.tile([128, 8, D], F16, tag="wp")
            nc.sync.dma_start(out=wp_sb, in_=r3(wp_e.ap(), 8, D))

            # ---------------- Phase 0/1: load inputs, qkv matmuls ----------
            with tc.tile_pool(name="ph1", bufs=1) as ph1, \
                 tc.tile_pool(name="ps1", bufs=4, space="PSUM") as ps1:

                xT = ph1.tile([128, 8, M], F16, tag="xT")
                nc.sync.dma_start(out=xT, in_=r3(xT_e.ap(), 8, M))
                wq = ph1.tile([128, 8, D], F16, tag="wq")
                nc.sync.dma_start(out=wq, in_=r3(wq_e.ap(), 8, D))
                wk = ph1.tile([128, 8, D], F16, tag="wk")
                nc.sync.dma_start(out=wk, in_=r3(wk_e.ap(), 8, D))
                wv = ph1.tile([128, 8, D], F16, tag="wv")
                nc.sync.dma_start(out=wv, in_=r3(wv_e.ap(), 8, D))

                # K: feature-major [kv-dim strip s][128, 2048]
                for s in range(8):
                    for mc in range(4):
                        pt = ps1.tile([128, 512], F32, tag="ps1")
                        for dt in range(8):
                            nc.tensor.matmul(
                                pt, lhsT=wk[:, dt, s * 128:(s + 1) * 128],
                                rhs=xT[:, dt, mc * 512:(mc + 1) * 512],
                                start=(dt == 0), stop=(dt == 7))
                        nc.scalar.mul(k_s[s][:, mc * 512:(mc + 1) * 512],
                                      pt, 1.0 / GRID)

                # Q: feature-major, tokens 0..1023 of rotated xT
                for s in range(8):
                    for mc in range(2):
                        pt = ps1.tile([128, 512], F32, tag="ps1")
                        for dt in range(8):
                            nc.tensor.matmul(
                                pt, lhsT=wq[:, dt, s * 128:(s + 1) * 128],
                                rhs=xT[:, dt, mc * 512:(mc + 1) * 512],
                                start=(dt == 0), stop=(dt == 7))
                        nc.scalar.mul(q_s[s][:, mc * 512:(mc + 1) * 512],
                                      pt, 1.0 / GRID)

                # V: token-major [tok strip ts][128, 1024]
                for ts in range(16):
                    for cc in range(2):
                        pt = ps1.tile([128, 512], F32, tag="ps1")
                        for dt in range(8):
                            nc.tensor.matmul(
                                pt, lhsT=xT[:, dt, ts * 128:(ts + 1) * 128],
                                rhs=wv[:, dt, cc * 512:(cc + 1) * 512],
                                start=(dt == 0), stop=(dt == 7))
                        nc.scalar.mul(v_t[ts][:, cc * 512:(cc + 1) * 512],
                                      pt, 1.0 / GRID)

            # ---------------- Phase 2: attention ---------------------------
            NCH = 2   # n-chunks of 512
            aop = ctx.enter_context(tc.tile_pool(name="aop", bufs=1))
            ao_s = [aop.tile([128, NQ], F16, tag=f"ao{s}", name=f"ao{s}")
                    for s in range(8)]
            with tc.tile_pool(name="expp", bufs=2) as expp, \
                 tc.tile_pool(name="attn", bufs=2) as attnp, \
                 tc.tile_pool(name="rbp", bufs=1) as rbp, \
                 tc.tile_pool(name="dots", bufs=1, space="PSUM") as dotsp, \
                 tc.tile_pool(name="spsum", bufs=1, space="PSUM") as spsum, \
                 tc.tile_pool(name="avps", bufs=1, space="PSUM") as avps:

                for hp in range(HP):
                    for ch in range(NCH):
                        n0 = ch * 512
                        # --- dots + exp, 2 m-tiles per PSUM supertile;
                        #     S: slabs < NS_PE accumulate [128,512] via
                        #     ones[128,128] matmul; the rest via DVE fold ---
                        exp_b = [expp.tile([128, 16, 512], F16, tag=f"exp{h}", name=f"exp{h}")
                                 for h in range(2)]
                        S_ps = [spsum.tile([128, 512], F32, tag=f"S{h}", name=f"S{h}")
                                for h in range(2)]
                        for g in range(8):
                            dt_ps = [dotsp.tile([128, 2, 512], F32, tag=f"dots{h}", name=f"dots{h}")
                                     for h in range(2)]
                            for sub in range(2):
                                mt = 2 * g + sub
                                for h in range(2):
                                    p0 = h * 64
                                    nc.tensor.matmul(
                                        dt_ps[h][:, sub, :],
                                        lhsT=k_s[hp][p0:p0 + 64,
                                                     mt * 128:(mt + 1) * 128],
                                        rhs=q_s[hp][p0:p0 + 64, n0:n0 + 512],
                                        start=True, stop=True,
                                        tile_position=(p0, 0))
                            for h in range(2):
                                nc.scalar.activation(
                                    exp_b[h][:, 2 * g:2 * g + 2, :],
                                    dt_ps[h][:, :, :], AF.Exp, scale=0.125)
                            for sub in range(2):
                                mt = 2 * g + sub
                                if mt >= NS_PE:
                                    continue
                                for h in range(2):
                                    nc.tensor.matmul(
                                        S_ps[h], lhsT=ones2d,
                                        rhs=exp_b[h][:, mt, :],
                                        start=(mt == 0), stop=False)

                        # --- fold slabs NS_PE..15 on DVE, add via one more
                        #     accumulating ones-matmul; rb = 4096/S f16 ---
                        rb = []
                        nfold = 16 - NS_PE
                        for h in range(2):
                            sf = rbp.tile([128, 512], F16, tag=f"sf{h}",
                                          name=f"sf{h}")
                            fold_view = bass.AP(
                                tensor=exp_b[h].tensor,
                                offset=exp_b[h].offset + NS_PE * 512,
                                ap=[exp_b[h].ap[0], [1, 512], [512, nfold]])
                            with nc.allow_low_precision(reason="S partial in f16; validated"):
                                nc.vector.tensor_reduce(
                                    sf, fold_view, mybir.AxisListType.X, ALU.add)
                            nc.tensor.matmul(S_ps[h], lhsT=ones2d, rhs=sf,
                                             start=False, stop=True)
                            rf = rbp.tile([128, 512], F32, tag=f"rf{h}")
                            nc.vector.reciprocal_approx_fast(rf, S_ps[h])
                            rbt = rbp.tile([128, 512], F16, tag=f"rb{h}")
                            nc.scalar.mul(rbt, rf, GRID)
                            rb.append(rbt)

                        # --- attn = floor(exp*4096/S) -> fp16; attn @ v ---
                        av0 = avps.tile([128, 512], F32, tag="av0", bufs=1)
                        av1 = avps.tile([128, 512], F32, tag="av1", bufs=1)
                        avt = [av0, av1]
                        for g in range(4):
                            for h in range(2):
                                y = attnp.tile([128, 4, 512], F16, tag="y", bufs=1)
                                nc.vector.tensor_tensor(
                                    y, exp_b[h][:, 4 * g:4 * g + 4, :],
                                    _rep_free(rb[h][:, :], 4), ALU.mult)
                                ai = attnp.tile([128, 4, 512], I16, tag="ai",
                                                bufs=1)
                                nc.vector.tensor_scalar(
                                    ai, y, -0.499, None, op0=ALU.add)
                                af = attnp.tile([128, 4, 512], F16, tag="af")
                                if h == 0:
                                    nc.vector.tensor_copy(af, ai)
                                else:
                                    nc.scalar.copy(af, ai)
                                for sub in range(4):
                                    mt = 4 * g + sub
                                    p0 = h * 64
                                    nc.tensor.matmul(
                                        avt[h][p0:p0 + 64, :],
                                        lhsT=v_t[mt][:, (2 * hp + h) * 64:
                                                     (2 * hp + h + 1) * 64],
                                        rhs=af[:, sub, :],
                                        start=(mt == 0), stop=(mt == 15),
                                        tile_position=(0, p0))
                        # exact fxp trunc for attn@v output (values ~90 grid
                        # units; truncation pattern must match the reference;
                        # |av| < 2 so no clip needed)
                        for h, av in ((0, av0), (1, av1)):
                            p0, p1 = h * 64, h * 64 + 64
                            src = av[p0:p1, :]
                            dst = ao_s[hp][p0:p1, n0:n0 + 512]
                            sg_t = attnp.tile([128, 512], F16, tag="fx_s",
                                              name="sg_t")
                            sg = sg_t[p0:p1, :]
                            nc.scalar.activation(sg, src, AF.Sign)
                            gi_t = attnp.tile([128, 512], I16, tag="fx_i",
                                              name="gi_t")
                            gi = gi_t[p0:p1, :]
                            nc.vector.scalar_tensor_tensor(
                                gi, in0=sg, scalar=-0.499, in1=src,
                                op0=ALU.mult, op1=ALU.add)
                            nc.vector.tensor_scalar(dst, gi, 1.0 / GRID, None,
                                                    op0=ALU.mult)

            # ---------------- Phase 3: projection --------------------------
            with tc.tile_pool(name="ps3", bufs=2, space="PSUM") as ps3, \
                 tc.tile_pool(name="outp", bufs=2) as outp:
                out3 = r3(out_e.ap(), 8, NQ)
                for ds in range(8):
                    for ch in range(2):
                        pt = ps3.tile([128, 512], F32, tag="ps3")
                        for es in range(8):
                            nc.tensor.matmul(
                                pt, lhsT=wp_sb[:, es, ds * 128:(ds + 1) * 128],
                                rhs=ao_s[es][:, ch * 512:(ch + 1) * 512],
                                start=(es == 0), stop=(es == 7))
                        t = outp.tile([128, 512], F32, tag="ot")
                        nc.vector.tensor_scalar(t, pt, bias_sb[:, ds:ds + 1],
                                                None, op0=ALU.add)
                        sg = outp.tile([128, 512], F16, tag="os")
                        nc.scalar.activation(sg, t, AF.Sign)
                        gi = outp.tile([128, 512], I16, tag="oi")
                        nc.vector.scalar_tensor_tensor(
                            gi, in0=sg, scalar=-0.499, in1=t,
                            op0=ALU.mult, op1=ALU.add)
                        nc.sync.dma_start(
                            out=out3[:, ds, ch * 512:(ch + 1) * 512], in_=gi)

    nc.compile()
    return nc


def _get_nc():
    global _CACHED_NC
    if _CACHED_NC is None:
        _CACHED_NC = build_kernel()
    return _CACHED_NC


def _pack_w(wT, col0):
    """wqkvT[:, col0:col0+1024] fp32 [1024, 1024] -> [128, 8, 1024] f16."""
    w = wT[:, col0:col0 + D]                        # [1024 dt*128+p, 1024 j]
    return np.ascontiguousarray(
        w.reshape(8, 128, D).transpose(1, 0, 2)).astype(np.float16)


def kernel(x, w_qkv, w_proj, b_proj, **_):
    B, N, Dm = x.shape
    assert (B, N, Dm) == (4, 2048, 1024)
    nc = _get_nc()

    wqkvT = np.ascontiguousarray((w_qkv.astype(np.float32) * GRID).T)
    wprojT = np.ascontiguousarray((w_proj.astype(np.float32) * GRID).T)
    wq = _pack_w(wqkvT, 0).reshape(128, 8 * D)
    wk = _pack_w(wqkvT, D).reshape(128, 8 * D)
    wv = _pack_w(wqkvT, 2 * D).reshape(128, 8 * D)
    wp = _pack_w(wprojT, 0).reshape(128, 8 * D)
    bias = np.ascontiguousarray(
        (b_proj.astype(np.float32) * GRID).reshape(8, 128).T)

    in_maps = []
    for c in range(8):
        b, t = c // 2, c % 2
        xb = x[b].astype(np.float32)
        xrot = np.concatenate([xb[t * NQ:], xb[:t * NQ]], axis=0)
        xT = np.ascontiguousarray(
            xrot.T.reshape(8, 128, M).transpose(1, 0, 2)
        ).astype(np.float16).reshape(128, 8 * M)
        in_maps.append({
            "xT": xT, "wq": wq, "wk": wk, "wv": wv, "wp": wp, "bias": bias,
        })

    res = run_bass_kernel_spmd(nc, in_maps, list(range(8)))
    global LAST_RESULT
    LAST_RESULT = res
    out = np.empty((B, N, Dm), dtype=np.float32)
    for c in range(8):
        b, t = c // 2, c % 2
        o = res.results[c]["out"].reshape(128, 8, NQ).transpose(1, 0, 2)
        out[b, t * NQ:(t + 1) * NQ, :] = o.reshape(D, NQ).T.astype(np.float32) / GRID
    return out


# revision 17
# speedup vs baseline: 2.7720x; 1.4103x over previous
"""Fixed-point attention (nn_Attention_17918603559191) on 8 TRN2 NeuronCores.

Sharding (no collectives): core c -> batch b = c//2, token-half t = c%2.
Each core computes K,V for all 2048 tokens of its batch (25% redundant
FLOPs on the qkv stage) and Q/attention/proj for its 1024 tokens. The
host rotates the token axis of x per core so that the core's q-tokens
are always columns 0..1023 (keeps the SPMD program identical across
cores); attention is invariant to permuting the key/value axis.

v2 numerics/pipeline (validated vs reference, rel ~9e-3):
 - host pre-converts all inputs to fp16 (weights pre-scaled by 4096 so
   PSUM results are in grid units); single large DMA per tensor
 - q/k/v: PSUM -> fp16 via one ACT Copy(scale=1/4096); the exact
   4096-grid truncation is skipped (validated: |q|max < 6 so no clip,
   trunc-vs-round differences are ~2e-4 relative and wash out)
 - S = sum_m exp: mt-slabs 0..NS_PE-1 accumulate via ones[128,128]
   matmul into PSUM; remaining slabs are mt-folded on DVE
   (tensor_reduce over a strided view) and folded into the same PSUM
   with one more accumulating ones-matmul. 4096/S via
   reciprocal_approx_fast (fp32) + ACT scale-copy.
 - attn = floor(exp * 4096/S): DVE tensor_tensor (y, f16), DVE
   tensor_scalar -0.499 -> int16 (round-to-nearest == floor), then the
   int16 -> fp16 cast split between DVE (h=0) and ACT (h=1) to balance
   engine load. No GPSIMD.
 - output returned as int16 grid units; host divides by 4096
"""

import sys

sys.path.insert(0, "/opt/trn_rl_repo")

import numpy as np
import concourse.bass as bass
import concourse.tile as tile
from concourse import mybir, bacc
from concourse.bass_utils import run_bass_kernel_spmd

F32 = mybir.dt.float32
F16 = mybir.dt.float16
I16 = mybir.dt.int16
ALU = mybir.AluOpType
AF = mybir.ActivationFunctionType

D = 1024      # model dim
M = 2048      # key/value tokens per core (full batch)
NQ = 1024     # query tokens per core
H = 16
DH = 64
HP = H // 2   # head pairs
GRID = 4096.0
NS_PE = 16    # mt-slabs of the softmax denominator summed on PE (rest DVE
              # via contiguous pairwise adds; 16 = all on PE)

_CACHED_NC = None


def _rep_free(ap, g):
    """Repeat a [P, W] AP g times along a new middle free dim (stride 0)."""
    return bass.AP(tensor=ap.tensor, offset=ap.offset,
                   ap=[ap.ap[0], [0, g], ap.ap[1]])


def build_kernel():
    nc = bacc.Bacc(name="fxp_attn")
    # all activations/weights fp16, host-prepared layouts:
    #   xT   [128, 8, 2048]  xT[p, dt, m] = x_rot[m, dt*128+p] (grid-neutral)
    #   wq   [128, 8, 1024]  wq[p, dt, j] = 4096*w_qkv[j, dt*128+p]
    #   wk   [128, 8, 1024]  cols 1024:2048 of qkv
    #   wv   [128, 8, 1024]  cols 2048:3072
    #   wp   [128, 8, 1024]  wp[p, es, j] = 4096*w_proj[j, es*128+p]
    #   bias [128, 8] f32    bias[p, ds] = 4096*b_proj[ds*128+p]
    xT_e = nc.declare_dram_parameter("xT", [128, 8 * M], F16, isOutput=False)
    wq_e = nc.declare_dram_parameter("wq", [128, 8 * D], F16, isOutput=False)
    wk_e = nc.declare_dram_parameter("wk", [128, 8 * D], F16, isOutput=False)
    wv_e = nc.declare_dram_parameter("wv", [128, 8 * D], F16, isOutput=False)
    wp_e = nc.declare_dram_parameter("wp", [128, 8 * D], F16, isOutput=False)
    bias_e = nc.declare_dram_parameter("bias", [128, 8], F32, isOutput=False)
    # out[p, ds*1024 + n] = int16 grid value of out[ds*128+p, n]
    out_e = nc.declare_dram_parameter("out", [128, 8 * NQ], I16, isOutput=True)

    def r3(ap, a, b):
        """View a [128, a*b] DRAM AP as [128, a, b]."""
        return bass.AP(tensor=ap.tensor, offset=0,
                       ap=[ap.ap[0], [b, a], [1, b]])

    with tile.TileContext(nc) as tc:
        from contextlib import ExitStack
        with ExitStack() as ctx:
            persist = ctx.enter_context(tc.tile_pool(name="persist", bufs=1))

            ones2d = persist.tile([128, 128], F16, tag="ones2d")
            nc.vector.memset(ones2d, 1.0)
            bias_sb = persist.tile([128, 8], F32, tag="bias")
            nc.sync.dma_start(out=bias_sb, in_=bias_e.ap())

            # persistent fp16 activations
            k_s = [persist.tile([128, M], F16, tag=f"k{s}", name=f"k{s}") for s in range(8)]
            q_s = [persist.tile([128, NQ], F16, tag=f"q{s}", name=f"q{s}") for s in range(8)]
            v_t = [persist.tile([128, D], F16, tag=f"v{t}", name=f"v{t}") for t in range(16)]
            wp_sb = persist.tile([128, 8, D], F16, tag="wp")
            nc.sync.dma_start(out=wp_sb, in_=r3(wp_e.ap(), 8, D))

            # ---------------- Phase 0/1: load inputs, qkv matmuls ----------
            with tc.tile_pool(name="ph1", bufs=1) as ph1, \
                 tc.tile_pool(name="ps1", bufs=4, space="PSUM") as ps1:

                xT = ph1.tile([128, 8, M], F16, tag="xT")
                nc.sync.dma_start(out=xT, in_=r3(xT_e.ap(), 8, M))
                wq = ph1.tile([128, 8, D], F16, tag="wq")
                nc.sync.dma_start(out=wq, in_=r3(wq_e.ap(), 8, D))
                wk = ph1.tile([128, 8, D], F16, tag="wk")
                nc.sync.dma_start(out=wk, in_=r3(wk_e.ap(), 8, D))
                wv = ph1.tile([128, 8, D], F16, tag="wv")
                nc.sync.dma_start(out=wv, in_=r3(wv_e.ap(), 8, D))

                # K: feature-major [kv-dim strip s][128, 2048]
                for s in range(8):
                    for mc in range(4):
                        pt = ps1.tile([128, 512], F32, tag="ps1")
                        for dt in range(8):
                            nc.tensor.matmul(
                                pt, lhsT=wk[:, dt, s * 128:(s + 1) * 128],
                                rhs=xT[:, dt, mc * 512:(mc + 1) * 512],
                                start=(dt == 0), stop=(dt == 7))
                        nc.scalar.mul(k_s[s][:, mc * 512:(mc + 1) * 512],
                                      pt, 1.0 / GRID)

                # Q: feature-major, tokens 0..1023 of rotated xT
                for s in range(8):
                    for mc in range(2):
                        pt = ps1.tile([128, 512], F32, tag="ps1")
                        for dt in range(8):
                            nc.tensor.matmul(
                                pt, lhsT=wq[:, dt, s * 128:(s + 1) * 128],
                                rhs=xT[:, dt, mc * 512:(mc + 1) * 512],
                                start=(dt == 0), stop=(dt == 7))
                        nc.scalar.mul(q_s[s][:, mc * 512:(mc + 1) * 512],
                                      pt, 1.0 / GRID)

                # V: token-major [tok strip ts][128, 1024]
                for ts in range(16):
                    for cc in range(2):
                        pt = ps1.tile([128, 512], F32, tag="ps1")
                        for dt in range(8):
                            nc.tensor.matmul(
                                pt, lhsT=xT[:, dt, ts * 128:(ts + 1) * 128],
                                rhs=wv[:, dt, cc * 512:(cc + 1) * 512],
                                start=(dt == 0), stop=(dt == 7))
                        nc.scalar.mul(v_t[ts][:, cc * 512:(cc + 1) * 512],
                                      pt, 1.0 / GRID)

            # ---------------- Phase 2: attention ---------------------------
            NCH = 2   # n-chunks of 512
            aop = ctx.enter_context(tc.tile_pool(name="aop", bufs=1))
            ao_s = [aop.tile([128, NQ], F16, tag=f"ao{s}", name=f"ao{s}")
                    for s in range(8)]
            with tc.tile_pool(name="expp", bufs=2) as expp, \
                 tc.tile_pool(name="attn", bufs=2) as attnp, \
                 tc.tile_pool(name="rbp", bufs=1) as rbp, \
                 tc.tile_pool(name="dots", bufs=1, space="PSUM") as dotsp, \
                 tc.tile_pool(name="spsum", bufs=1, space="PSUM") as spsum, \
                 tc.tile_pool(name="avps", bufs=1, space="PSUM") as avps:

                for hp in range(HP):
                    for ch in range(NCH):
                        n0 = ch * 512
                        # --- dots + exp, 2 m-tiles per PSUM supertile;
                        #     S: slabs < NS_PE accumulate [128,512] via
                        #     ones[128,128] matmul; the rest via DVE fold ---
                        exp_b = [expp.tile([128, 16, 512], F16, tag=f"exp{h}", name=f"exp{h}")
                                 for h in range(2)]
                        S_ps = [spsum.tile([128, 512], F32, tag=f"S{h}", name=f"S{h}")
                                for h in range(2)]
                        for g in range(8):
                            dt_ps = [dotsp.tile([128, 2, 512], F32, tag=f"dots{h}", name=f"dots{h}")
                                     for h in range(2)]
                            for sub in range(2):
                                mt = 2 * g + sub
                                for h in range(2):
                                    p0 = h * 64
                                    nc.tensor.matmul(
                                        dt_ps[h][:, sub, :],
                                        lhsT=k_s[hp][p0:p0 + 64,
                                                     mt * 128:(mt + 1) * 128],
                                        rhs=q_s[hp][p0:p0 + 64, n0:n0 + 512],
                                        start=True, stop=True,
                                        tile_position=(p0, 0))
                            for h in range(2):
                                nc.scalar.activation(
                                    exp_b[h][:, 2 * g:2 * g + 2, :],
                                    dt_ps[h][:, :, :], AF.Exp, scale=0.125)
                            for sub in range(2):
                                mt = 2 * g + sub
                                if mt >= NS_PE:
                                    continue
                                for h in range(2):
                                    nc.tensor.matmul(
                                        S_ps[h], lhsT=ones2d,
                                        rhs=exp_b[h][:, mt, :],
                                        start=(mt == 0),
                                        stop=(mt == NS_PE - 1 and NS_PE == 16))

                        # --- fold slabs NS_PE..15 on DVE via contiguous
                        #     pairwise adds, fold in with one more
                        #     accumulating ones-matmul; rb = 4096/S f16 ---
                        rb = []
                        nfold = 16 - NS_PE
                        for h in range(2):
                            if nfold:
                                sf = rbp.tile([128, 512], F16, tag=f"sf{h}",
                                              name=f"sf{h}")
                                eb = exp_b[h]
                                with nc.allow_low_precision(reason="S partial in f16; validated"):
                                    # tree-add slabs NS_PE..15 into sf
                                    nc.vector.tensor_tensor(
                                        sf, eb[:, NS_PE, :], eb[:, NS_PE + 1, :],
                                        ALU.add)
                                    for mt in range(NS_PE + 2, 16):
                                        nc.vector.tensor_tensor(
                                            sf, sf, eb[:, mt, :], ALU.add)
                                nc.tensor.matmul(S_ps[h], lhsT=ones2d, rhs=sf,
                                                 start=False, stop=True)
                            rf = rbp.tile([128, 512], F32, tag=f"rf{h}")
                            nc.vector.reciprocal_approx_fast(rf, S_ps[h])
                            rbt = rbp.tile([128, 512], F16, tag=f"rb{h}")
                            nc.scalar.mul(rbt, rf, GRID)
                            rb.append(rbt)

                        # --- attn = floor(exp*4096/S) -> fp16; attn @ v ---
                        av0 = avps.tile([128, 512], F32, tag="av0", bufs=1)
                        av1 = avps.tile([128, 512], F32, tag="av1", bufs=1)
                        avt = [av0, av1]
                        for g in range(4):
                            for h in range(2):
                                y = attnp.tile([128, 4, 512], F16, tag="y", bufs=1)
                                nc.vector.tensor_tensor(
                                    y, exp_b[h][:, 4 * g:4 * g + 4, :],
                                    _rep_free(rb[h][:, :], 4), ALU.mult)
                                ai = attnp.tile([128, 4, 512], I16, tag="ai",
                                                bufs=1)
                                nc.vector.tensor_scalar(
                                    ai, y, -0.499, None, op0=ALU.add)
                                af = attnp.tile([128, 4, 512], F16, tag="af")
                                nc.vector.tensor_copy(af, ai)
                                for sub in range(4):
                                    mt = 4 * g + sub
                                    p0 = h * 64
                                    nc.tensor.matmul(
                                        avt[h][p0:p0 + 64, :],
                                        lhsT=v_t[mt][:, (2 * hp + h) * 64:
                                                     (2 * hp + h + 1) * 64],
                                        rhs=af[:, sub, :],
                                        start=(mt == 0), stop=(mt == 15),
                                        tile_position=(0, p0))
                        # exact fxp trunc for attn@v output (values ~90 grid
                        # units; truncation pattern must match the reference;
                        # |av| < 2 so no clip needed)
                        for h, av in ((0, av0), (1, av1)):
                            p0, p1 = h * 64, h * 64 + 64
                            src = av[p0:p1, :]
                            dst = ao_s[hp][p0:p1, n0:n0 + 512]
                            sg_t = attnp.tile([128, 512], F16, tag="fx_s",
                                              name="sg_t")
                            sg = sg_t[p0:p1, :]
                            nc.scalar.activation(sg, src, AF.Sign)
                            gi_t = attnp.tile([128, 512], I16, tag="fx_i",
                                              name="gi_t")
                            gi = gi_t[p0:p1, :]
                            nc.vector.scalar_tensor_tensor(
                                gi, in0=sg, scalar=-0.499, in1=src,
                                op0=ALU.mult, op1=ALU.add)
                            nc.vector.tensor_scalar(dst, gi, 1.0 / GRID, None,
                                                    op0=ALU.mult)

            # ---------------- Phase 3: projection --------------------------
            with tc.tile_pool(name="ps3", bufs=2, space="PSUM") as ps3, \
                 tc.tile_pool(name="outp", bufs=2) as outp:
                out3 = r3(out_e.ap(), 8, NQ)
                for ds in range(8):
                    for ch in range(2):
                        pt = ps3.tile([128, 512], F32, tag="ps3")
                        for es in range(8):
                            nc.tensor.matmul(
                                pt, lhsT=wp_sb[:, es, ds * 128:(ds + 1) * 128],
                                rhs=ao_s[es][:, ch * 512:(ch + 1) * 512],
                                start=(es == 0), stop=(es == 7))
                        t = outp.tile([128, 512], F32, tag="ot")
                        nc.vector.tensor_scalar(t, pt, bias_sb[:, ds:ds + 1],
                                                None, op0=ALU.add)
                        sg = outp.tile([128, 512], F16, tag="os")
                        nc.scalar.activation(sg, t, AF.Sign)
                        gi = outp.tile([128, 512], I16, tag="oi")
                        nc.vector.scalar_tensor_tensor(
                            gi, in0=sg, scalar=-0.499, in1=t,
                            op0=ALU.mult, op1=ALU.add)
                        nc.sync.dma_start(
                            out=out3[:, ds, ch * 512:(ch + 1) * 512], in_=gi)

    nc.compile()
    return nc


def _get_nc():
    global _CACHED_NC
    if _CACHED_NC is None:
        _CACHED_NC = build_kernel()
    return _CACHED_NC


def _pack_w(wT, col0):
    """wqkvT[:, col0:col0+1024] fp32 [1024, 1024] -> [128, 8, 1024] f16."""
    w = wT[:, col0:col0 + D]                        # [1024 dt*128+p, 1024 j]
    return np.ascontiguousarray(
        w.reshape(8, 128, D).transpose(1, 0, 2)).astype(np.float16)


def kernel(x, w_qkv, w_proj, b_proj, **_):
    B, N, Dm = x.shape
    assert (B, N, Dm) == (4, 2048, 1024)
    nc = _get_nc()

    wqkvT = np.ascontiguousarray((w_qkv.astype(np.float32) * GRID).T)
    wprojT = np.ascontiguousarray((w_proj.astype(np.float32) * GRID).T)
    wq = _pack_w(wqkvT, 0).reshape(128, 8 * D)
    wk = _pack_w(wqkvT, D).reshape(128, 8 * D)
    wv = _pack_w(wqkvT, 2 * D).reshape(128, 8 * D)
    wp = _pack_w(wprojT, 0).reshape(128, 8 * D)
    bias = np.ascontiguousarray(
        (b_proj.astype(np.float32) * GRID).reshape(8, 128).T)

    in_maps = []
    for c in range(8):
        b, t = c // 2, c % 2
        xb = x[b].astype(np.float32)
        xrot = np.concatenate([xb[t * NQ:], xb[:t * NQ]], axis=0)
        xT = np.ascontiguousarray(
            xrot.T.reshape(8, 128, M).transpose(1, 0, 2)
        ).astype(np.float16).reshape(128, 8 * M)
        in_maps.append({
            "xT": xT, "wq": wq, "wk": wk, "wv": wv, "wp": wp, "bias": bias,
        })

    res = run_bass_kernel_spmd(nc, in_maps, list(range(8)))
    global LAST_RESULT
    LAST_RESULT = res
    out = np.empty((B, N, Dm), dtype=np.float32)
    for c in range(8):
        b, t = c // 2, c % 2
        o = res.results[c]["out"].reshape(128, 8, NQ).transpose(1, 0, 2)
        out[b, t * NQ:(t + 1) * NQ, :] = o.reshape(D, NQ).T.astype(np.float32) / GRID
    return out
